# revision 1
# baseline (speedup 1.0000x reference)
"""Sparse multi-head attention (ViT-style, 577 tokens, 12 heads) on 8 TRN2
NeuronCores.

Sharding: pure data-parallel over batch. Each core gets 8 of the 64 batch
items: 4 from the "large" half (full 12-head attention) and 4 from the
"small" half (compressed: heads 6..11 of q/k/v are statically zero, so only
6 heads + a 384x384 projection are computed). Co-sharding large/small
halves balances per-core compute. No collectives are needed.

Per-item dataflow (everything stays in the transposed domain so no
intermediate ever needs a device transpose except the initial x -> xT):

  x[577,768] --PE-transpose--> xT[c,n]
  qT,kT[o,n] = Wqkv^T-stationary matmuls over xT     (q pre-scaled by D^-0.5)
  v[n,o]     = xT-stationary matmuls over Wv^T, plus a ones column (aug)
  S^T[m,n]   = kT-stationary over qT (per head, K=64)
  P^T        = exp(S^T)  (scalar engine, PSUM->SBUF, bf16; softmax max-shift
               skipped: logits are O(1) by construction)
  aoT[d,n]   = v_aug^T @ P^T  -> row 64 holds the softmax denominators
  normalize via reciprocal + K=1 broadcast matmul
  y[n,oc]    = aoT-stationary over proj_w^T, + bias, DMA out.

Matmuls run as float32r (full PE rate at free-dim >= 256) on fp32 data;
only P^T and v are bf16 (flash-attention-style precision).
"""

import ml_dtypes
import numpy as np
from contextlib import ExitStack

import concourse.bass as bass
import concourse.tile as tile
from concourse import bacc, mybir
from concourse import bass2jax as _b2j
from concourse.bass_utils import run_bass_kernel_spmd
from concourse.masks import make_identity


def _run_bass_via_pjrt_presharded(nc, in_maps, n_cores):
    """Drop-in replacement for bass2jax.run_bass_via_pjrt (multi-core path).

    The stock version concatenates per-core inputs into one host array and
    lets jax reshard it onto the mesh; on the neuron PJRT backend that
    resharding lowers to a compiled "scatter" program which, for ~100MB
    inputs, dies in neuronx-cc codegen (16-bit semaphore_wait_value
    overflow). Here each per-core shard is device_put directly onto its
    device and the global array is assembled zero-copy, so the jitted body
    sees correctly-sharded operands and no data-movement program exists.
    """
    import jax

    _b2j.install_neuronx_cc_hook()
    assert nc.dbg_addr is None and nc.partition_id_tensor is None

    from jax.experimental.shard_map import shard_map
    from jax.sharding import Mesh, NamedSharding, PartitionSpec

    in_names, out_names, out_avals, zero_shapes = [], [], [], []
    for alloc in nc.m.functions[0].allocations:
        if not isinstance(alloc, mybir.MemoryLocationSet):
            continue
        name = alloc.memorylocations[0].name
        if alloc.kind == "ExternalInput":
            in_names.append(name)
        elif alloc.kind == "ExternalOutput":
            shape = tuple(alloc.tensor_shape)
            dtype = mybir.dt.np(alloc.dtype)
            out_names.append(name)
            out_avals.append(jax.core.ShapedArray(shape, dtype))
            zero_shapes.append((shape, dtype))
    n_params = len(in_names)
    n_outs = len(out_names)
    all_names = in_names + out_names
    donate = tuple(range(n_params, n_params + n_outs))

    def _body(*args):
        outs = _b2j._bass_exec_p.bind(
            *args,
            out_avals=tuple(out_avals),
            in_names=tuple(all_names),
            out_names=tuple(out_names),
            lowering_input_output_aliases=(),
            sim_require_finite=True,
            sim_require_nnan=True,
            nc=nc,
        )
        return tuple(outs)

    devices = jax.devices()[:n_cores]
    mesh = Mesh(np.asarray(devices), ("core",))
    sharding = NamedSharding(mesh, PartitionSpec("core"))

    def make_global(shards):
        s0 = np.asarray(shards[0])
        gshape = (n_cores * s0.shape[0], *s0.shape[1:])
        parts = [
            jax.device_put(np.ascontiguousarray(shards[c]), devices[c])
            for c in range(n_cores)
        ]
        return jax.make_array_from_single_device_arrays(gshape, sharding, parts)

    global_ins = [make_global([m[nm] for m in in_maps]) for nm in in_names]
    global_zeros = [
        make_global([np.zeros(shape, dtype)] * n_cores)
        for shape, dtype in zero_shapes
    ]
    # force H2D completion so the NEFF's DMAs don't contend with PCIe-in
    for a in (*global_ins, *global_zeros):
        jax.block_until_ready(a)

    sharded = jax.jit(
        shard_map(_body, mesh=mesh, in_specs=(PartitionSpec("core"),) * (n_params + n_outs),
                  out_specs=(PartitionSpec("core"),) * n_outs, check_rep=False),
        donate_argnums=donate,
        keep_unused=True,
    )
    out_arrs = sharded(*global_ins, *global_zeros)

    results = [dict() for _ in range(n_cores)]
    for i, name in enumerate(out_names):
        arr = out_arrs[i]
        per = {s.index[0].start or 0: np.asarray(s.data) for s in arr.addressable_shards}
        step = out_avals[i].shape[0]
        for c in range(n_cores):
            results[c][name] = per[c * step]
    return results


def _patched_run_bass_via_pjrt(nc, in_maps, n_cores):
    if n_cores > 1 and nc.partition_id_tensor is None and nc.dbg_addr is None:
        return _run_bass_via_pjrt_presharded(nc, in_maps, n_cores)
    return _orig_run_bass_via_pjrt(nc, in_maps, n_cores)


_orig_run_bass_via_pjrt = _b2j.run_bass_via_pjrt
_b2j.run_bass_via_pjrt = _patched_run_bass_via_pjrt

P = 128
N = 577
C = 768
H = 12
D = 64
NCH = 5           # n (token) chunks: 4*128 + 65
CCH = 6           # c chunks: 768 / 128
NTAIL = N - 4 * P  # 65
F0, F1 = 290, 288  # n free-dim halves, padded n=578: fp32r needs EVEN free sizes
HALVES = ((0, F0), (F0, F1))
ITEMS = 8
NCORES = 8

f32 = mybir.dt.float32
f32r = mybir.dt.float32r
bf16 = mybir.dt.bfloat16


def _rows(nch):
    return NTAIL if nch == NCH - 1 else P


def _mcols(nch):
    """lhsT column count for an n-chunk: pad the 65-tail to 66 (even M is
    measurably faster on the PE); the extra output partition is discarded."""
    return NTAIL + 1 if nch == NCH - 1 else P


def _emit(ctx, tc, x_ext, wq_ext, pw_ext, pb_ext, sel_ext, out_ext):
    nc = tc.nc

    const_pool = ctx.enter_context(tc.tile_pool(name="const", bufs=1))
    wpool = ctx.enter_context(tc.tile_pool(name="weights", bufs=1))
    xtpool = ctx.enter_context(tc.tile_pool(name="xt", bufs=2))
    qkpool = ctx.enter_context(tc.tile_pool(name="qkt", bufs=2))
    vpool = ctx.enter_context(tc.tile_pool(name="vnat", bufs=2))
    epool = ctx.enter_context(tc.tile_pool(name="exps", bufs=4))
    aopool = ctx.enter_context(tc.tile_pool(name="aot", bufs=2))
    ypool = ctx.enter_context(tc.tile_pool(name="ychunk", bufs=2))
    spool = ctx.enter_context(tc.tile_pool(name="norm", bufs=2))
    aoupool = ctx.enter_context(tc.tile_pool(name="aou", bufs=2))
    # PSUM: 3x 2-bank slots (scores pairs + AV out) + 2x 1-bank slots
    # (qkv / proj / transpose / bcast staging) = 8 banks.
    ps2 = ctx.enter_context(tc.tile_pool(name="ps2", bufs=3, space="PSUM"))
    ps1 = ctx.enter_context(tc.tile_pool(name="ps1", bufs=2, space="PSUM"))

    def ps2_tile(name):
        return ps2.tile([P, 2, 512], f32, tag="ps2", name=name)

    def ps1_tile(name):
        return ps1.tile([P, 512], f32, tag="ps1", name=name)

    # ---- constants / weights (resident) ----
    ones_f32 = const_pool.tile([1, P], f32, name="ones_f32")
    nc.gpsimd.memset(ones_f32[:], 1.0)
    ones_row = const_pool.tile([1, P], f32r, name="ones_row")
    nc.vector.tensor_copy(ones_row[:], ones_f32[:])

    selA = const_pool.tile([6, CCH // 2, P], bf16, name="selA")
    nc.sync.dma_start(selA[:], sel_ext[0])
    selB = const_pool.tile([6, CCH // 2, P], bf16, name="selB")
    nc.sync.dma_start(selB[:], sel_ext[1])

    wq_sb = wpool.tile([P, CCH, 3 * C], bf16, name="wq_sb")
    nc.sync.dma_start(wq_sb[:], wq_ext.rearrange("(co p) o -> p co o", p=P))
    pw_sb = wpool.tile([P, CCH, C], bf16, name="pw_sb")
    nc.sync.dma_start(pw_sb[:], pw_ext.rearrange("(co p) o -> p co o", p=P))
    pb_sb = const_pool.tile([1, C], f32r, name="pb_sb")
    nc.sync.dma_start(pb_sb[:], pb_ext[None, :])

    # bias broadcast across partitions: [128, 768] = ones[128,1] @ pb[1,768]
    bias_sb = wpool.tile([P, C], f32, name="bias_sb")
    for j in range(2):
        psb0 = ps1_tile("ps_bias")
        nc.tensor.matmul(
            psb0[:, 0:384],
            lhsT=ones_row[0:1, :],
            rhs=pb_sb[0:1, j * 384:(j + 1) * 384],
            start=True, stop=True,
        )
        nc.vector.tensor_copy(bias_sb[:, j * 384:(j + 1) * 384], psb0[:, 0:384])

    # ---- per-item pipeline (1-item software pipeline for the tail) ----
    def emit_tail(st):
        aoT, aoU, drecips, it, small = st
        Heff = H // 2 if small else H
        CCH_ao = CCH // 2 if small else CCH
        NJ = 1 if small else 2
        for c in range(Heff // 2):
            hb, cw = divmod(c, CCH // 2)
            selh = selA if hb == 0 else selB
            drecip = drecips[hb]
            for j, (n0, nsz) in enumerate(HALVES):
                pbc = ps1_tile("ps_bc")
                nc.tensor.matmul(
                    pbc[:, 0:nsz],
                    lhsT=selh[0:6, cw, :],
                    rhs=drecip[0:6, j, 0:nsz],
                    start=True, stop=True,
                )
                nc.vector.tensor_mul(
                    aoT[:, c, n0:n0 + nsz],
                    aoU[:, c, j, 0:nsz],
                    pbc[:, 0:nsz],
                )
        for nch in range(NCH):
            rows = _rows(nch)
            yc = ypool.tile([P, C], f32, name="yc")
            if small:
                nc.gpsimd.memset(yc[0:rows, 384:768], 0.0)
            for j in range(NJ):
                o0 = j * 384
                psy = ps1_tile("ps_y")
                for cc in range(CCH_ao):
                    nc.tensor.matmul(
                        psy[0:_mcols(nch), 0:384],
                        lhsT=aoT[:, cc, nch * P:nch * P + _mcols(nch)],
                        rhs=pw_sb[:, cc, o0:o0 + 384],
                        start=(cc == 0), stop=(cc == CCH_ao - 1),
                    )
                nc.vector.tensor_add(
                    yc[0:rows, o0:o0 + 384],
                    psy[0:rows, 0:384],
                    bias_sb[0:rows, o0:o0 + 384],
                )
            nc.sync.dma_start(out_ext[it, nch * P:nch * P + rows, :],
                              yc[0:rows, :])

    pending = None
    for it in range(ITEMS):
        small = it >= ITEMS // 2
        Heff = H // 2 if small else H
        qk_chunks = ([0, 1, 2, 6, 7, 8] if small else list(range(12)))
        NJ = 1 if small else 2

        # Phase A: x arrives pre-transposed (and bf16) from the host
        xT = xtpool.tile([P, CCH, 640], bf16, name="xT")
        nc.gpsimd.memset(xT[:, :, N], 0.0)
        nc.sync.dma_start(
            xT[:, :, 0:N],
            x_ext[it].rearrange("(co p) n -> p co n", p=P))

        # Phase B: qT / kT (transposed outputs) for needed o-chunks
        qkT = qkpool.tile([P, 12, N + 1], bf16, name="qkT")
        for oc in qk_chunks:
            wcol = oc * P if oc < 6 else C + (oc - 6) * P
            for j, (n0, nsz) in enumerate(HALVES):
                pqk = ps1_tile("ps_qk")
                for cc in range(CCH):
                    nc.tensor.matmul(
                        pqk[:, 0:nsz],
                        lhsT=wq_sb[:, cc, wcol:wcol + P],
                        rhs=xT[:, cc, n0:n0 + nsz],
                        start=(cc == 0), stop=(cc == CCH - 1),
                    )
                nc.vector.tensor_copy(qkT[:, oc, n0:n0 + nsz], pqk[:, 0:nsz])

        # Phase C: v natural [n-part, (h,d)-free], 128-wide lanes (FWL),
        # ones column at d=64 feeds the softmax denominators.
        v_nat = vpool.tile([P, NCH, H, 2 * D], bf16, name="v_nat")
        nc.gpsimd.memset(v_nat[:, :, :, D:2 * D], 0.0)
        nc.gpsimd.memset(v_nat[:, :, :, D], 1.0)
        for nch in range(NCH):
            rows = _rows(nch)
            for j in range(NJ):
                vcol = 2 * C + j * 384
                pv = ps1_tile("ps_v")
                for cc in range(CCH):
                    nc.tensor.matmul(
                        pv[0:_mcols(nch), 0:384],
                        lhsT=xT[:, cc, nch * P:nch * P + _mcols(nch)],
                        rhs=wq_sb[:, cc, vcol:vcol + 384],
                        start=(cc == 0), stop=(cc == CCH - 1),
                    )
                nc.vector.tensor_copy(
                    v_nat[0:rows, nch, j * 6:(j + 1) * 6, 0:D],
                    pv[0:rows, 0:384].rearrange("p (h d) -> p h d", h=6),
                )

        # Phase D/E: 2-stage pair pipeline — scores+exp of pair p overlap
        # AV of pair p-1, so the scalar engine's exp stream stays ahead of
        # the PE's AV consumption.
        aoT = aopool.tile([P, CCH, N + 1], bf16, name="aoT")
        aoU = aoupool.tile([P, CCH, 2, F0], bf16, name="aoU")
        dsums = [spool.tile([6, 2, F0], f32, tag="dsum", name=f"dsum{hb}")
                 for hb in range(2 if Heff == H else 1)]
        drecips = [None, None]

        def emit_scores(hp):
            kch = 6 + hp
            expS = [None, None]
            for hh in (0, 1):
                expS[hh] = epool.tile([P, NCH, 2, F0], bf16, tag="expS",
                                      name=f"expS{hh}")
            for mch in range(NCH):
                mrows = _rows(mch)
                pse = [ps2_tile("ps_s0"), ps2_tile("ps_s1")]
                for j, (n0, nsz) in enumerate(HALVES):
                    for hh in (0, 1):
                        # adjacent issue of disjoint row-groups -> the two
                        # K=64 matmuls run concurrently in the PE array
                        hrow = hh * D
                        nc.tensor.matmul(
                            pse[hh][0:_mcols(mch), j, 0:nsz],
                            lhsT=qkT[hrow:hrow + D, kch, mch * P:mch * P + _mcols(mch)],
                            rhs=qkT[hrow:hrow + D, hp, n0:n0 + nsz],
                            start=True, stop=True,
                            tile_position=(hrow, 0),
                        )
                for hh in (0, 1):
                    nc.scalar.activation(
                        expS[hh][0:mrows, mch, :, :],
                        pse[hh][0:mrows, :, 0:F0],
                        mybir.ActivationFunctionType.Exp,
                    )
            return expS

        def emit_av(hp, expS):
            for hh in (0, 1):
                h = 2 * hp + hh
                po = ps2_tile("ps_o")
                for j, (n0, nsz) in enumerate(HALVES):
                    for mch in range(NCH):
                        mrows = _rows(mch)
                        nc.tensor.matmul(
                            po[:, j, 0:nsz],
                            lhsT=v_nat[0:mrows, mch, h, :],
                            rhs=expS[hh][0:mrows, mch, j, 0:nsz],
                            start=(mch == 0), stop=(mch == NCH - 1),
                        )
                arow = hh * D
                nc.vector.tensor_copy(aoU[arow:arow + D, hp, :, :],
                                      po[0:D, :, 0:F0])
                dstage = spool.tile([1, 2, F0], f32, name="dstage")
                nc.vector.tensor_copy(dstage[0:1, :, :], po[D:D + 1, :, 0:F0])
                hb, hw = divmod(h, 6)
                nc.gpsimd.dma_start(dsums[hb][hw:hw + 1, :, :],
                                    dstage[0:1, :, :])

        def emit_recip(hb):
            dr = spool.tile([6, 2, F0], bf16, tag="drecip", name=f"drecip{hb}")
            with nc.allow_low_precision(reason="softmax recip bcast via bf16 matmul"):
                nc.vector.reciprocal(dr[:, :, :], dsums[hb][:, :, :])
            drecips[hb] = dr

        prev = None
        for hp in range(Heff // 2):
            expS = emit_scores(hp)
            if hp == 0 and pending is not None:
                emit_tail(pending)      # prev item's normalize+proj: PE filler
                pending = None
            if prev is not None:
                emit_av(prev[0], prev[1])
                if prev[0] == 2:
                    emit_recip(0)       # heads 0..5 all staged
            prev = (hp, expS)
        emit_av(prev[0], prev[1])
        if Heff == H:
            emit_recip(1)
        else:
            emit_recip(0)
        pending = (aoT, aoU, drecips, it, small)

    emit_tail(pending)


_GRAPH = None


def _get_graph():
    global _GRAPH
    if _GRAPH is None:
        nc = bacc.Bacc("TRN2", target_bir_lowering=False, debug=False,
                       num_devices=NCORES)
        x_ext = nc.dram_tensor("x", [ITEMS, C, N], bf16, kind="ExternalInput").ap()
        wq_ext = nc.dram_tensor("wq", [C, 3 * C], bf16, kind="ExternalInput").ap()
        pw_ext = nc.dram_tensor("pw", [C, C], bf16, kind="ExternalInput").ap()
        pb_ext = nc.dram_tensor("pb", [C], f32r, kind="ExternalInput").ap()
        sel_ext = nc.dram_tensor("sel", [2, 6, CCH // 2, P], bf16, kind="ExternalInput").ap()
        out_ext = nc.dram_tensor("out", [ITEMS, N, C], f32, kind="ExternalOutput").ap()
        with tile.TileContext(nc) as tc:
            with ExitStack() as ctx:
                _emit(ctx, tc, x_ext, wq_ext, pw_ext, pb_ext, sel_ext, out_ext)
        nc.finalize()
        _GRAPH = nc
    return _GRAPH


LAST_RESULTS = None


def kernel(x, qkv_w, proj_w, proj_b, _trace=False):
    global LAST_RESULTS
    x = np.asarray(x, dtype=np.float32)
    xT_all = np.ascontiguousarray(x.transpose(0, 2, 1)).astype(ml_dtypes.bfloat16)
    wq = np.array(qkv_w, dtype=np.float32)          # copy; rows 0:C are q
    wq[0:C] *= D ** -0.5                            # fold attention scale into Wq
    wqT = np.ascontiguousarray(wq.T).astype(ml_dtypes.bfloat16)   # [C, 3C]
    pwT = np.ascontiguousarray(
        np.asarray(proj_w, dtype=np.float32).T).astype(ml_dtypes.bfloat16)
    pb = np.ascontiguousarray(np.asarray(proj_b, dtype=np.float32))
    sel_np = np.zeros((2, 6, CCH // 2, P), dtype=ml_dtypes.bfloat16)
    for c in range(CCH):
        hb, cw = divmod(c, CCH // 2)
        sel_np[hb, 2 * cw, cw, 0:D] = 1
        sel_np[hb, 2 * cw + 1, cw, D:P] = 1

    nc = _get_graph()
    in_maps = []
    half = x.shape[0] // 2  # 32
    per = half // NCORES    # 4
    for c in range(NCORES):
        xs = np.concatenate(
            [xT_all[per * c:per * (c + 1)],
             xT_all[half + per * c:half + per * (c + 1)]],
            axis=0,
        )
        in_maps.append({
            "x": np.ascontiguousarray(xs),
            "wq": wqT,
            "pw": pwT,
            "pb": pb,
            "sel": sel_np,
        })

    res = run_bass_kernel_spmd(nc, in_maps, core_ids=list(range(NCORES)),
                               trace=_trace)
    LAST_RESULTS = res

    out = np.empty((x.shape[0], N, C), dtype=np.float32)
    for c in range(NCORES):
        o = res.results[c]["out"]
        out[per * c:per * (c + 1)] = o[0:per]
        out[half + per * c:half + per * (c + 1)] = o[per:2 * per]
    return out



# revision 9
# speedup vs baseline: 1.1872x; 1.1872x over previous
"""Sparse multi-head attention (ViT-style, 577 tokens, 12 heads) on 8 TRN2
NeuronCores.

Sharding: pure data-parallel over batch. Each core gets 8 of the 64 batch
items: 4 from the "large" half (full 12-head attention) and 4 from the
"small" half (compressed: heads 6..11 of q/k/v are statically zero, so only
6 heads + a 384x384 projection are computed). Co-sharding large/small
halves balances per-core compute. No collectives are needed.

Per-item dataflow (everything stays in the transposed domain so no
intermediate ever needs a device transpose except the initial x -> xT):

  x[577,768] arrives pre-transposed + bf16 from host as xT[c,n]
  qT,kT[o,n] = Wqkv^T-stationary matmuls over xT     (q pre-scaled by D^-0.5)
  v[n,o]     = xT-stationary matmuls over Wv^T, plus a ones column (aug)
  S^T[m,n]   = kT-stationary over qT (per head, K=64, row-group pairs)
  P^T        = exp(S^T)  (scalar engine, PSUM->SBUF, bf16; softmax max-shift
               skipped: logits are O(1) by construction)
  aoT[d,n]   = v_aug^T @ P^T  -> row 64 holds the softmax denominators
  normalize via fast-approx reciprocal + K=6 broadcast matmul (f32r)
  y[n,oc]    = aoT-stationary over proj_w^T, + bias, DMA out.

Scheduling: the exp stream on the scalar engine is the per-phase bottleneck
of the softmax (D/E) phase, so the qkv-projection work (B/C) of item i+1 is
emitted as PE "filler" interleaved INTO item i's D/E phase - the PE chews
projection matmuls while waiting for exp results, instead of stalling and
re-paying weight-load latency on every resume. The tail (normalize+proj) of
item i-1 likewise folds into item i's first score phase.
"""

import ml_dtypes
import numpy as np
from collections import deque
from contextlib import ExitStack

import concourse.bass as bass
import concourse.tile as tile
from concourse import bacc, mybir
from concourse import bass2jax as _b2j
from concourse.bass_utils import run_bass_kernel_spmd


def _run_bass_via_pjrt_presharded(nc, in_maps, n_cores):
    """Drop-in replacement for bass2jax.run_bass_via_pjrt (multi-core path).

    The stock version concatenates per-core inputs into one host array and
    lets jax reshard it onto the mesh; on the neuron PJRT backend that
    resharding lowers to a compiled "scatter" program which, for ~100MB
    inputs, dies in neuronx-cc codegen (16-bit semaphore_wait_value
    overflow). Here each per-core shard is device_put directly onto its
    device and the global array is assembled zero-copy, so the jitted body
    sees correctly-sharded operands and no data-movement program exists.
    """
    import jax

    _b2j.install_neuronx_cc_hook()
    assert nc.dbg_addr is None and nc.partition_id_tensor is None

    from jax.experimental.shard_map import shard_map
    from jax.sharding import Mesh, NamedSharding, PartitionSpec

    in_names, out_names, out_avals, zero_shapes = [], [], [], []
    for alloc in nc.m.functions[0].allocations:
        if not isinstance(alloc, mybir.MemoryLocationSet):
            continue
        name = alloc.memorylocations[0].name
        if alloc.kind == "ExternalInput":
            in_names.append(name)
        elif alloc.kind == "ExternalOutput":
            shape = tuple(alloc.tensor_shape)
            dtype = mybir.dt.np(alloc.dtype)
            out_names.append(name)
            out_avals.append(jax.core.ShapedArray(shape, dtype))
            zero_shapes.append((shape, dtype))
    n_params = len(in_names)
    n_outs = len(out_names)
    all_names = in_names + out_names
    donate = tuple(range(n_params, n_params + n_outs))

    def _body(*args):
        outs = _b2j._bass_exec_p.bind(
            *args,
            out_avals=tuple(out_avals),
            in_names=tuple(all_names),
            out_names=tuple(out_names),
            lowering_input_output_aliases=(),
            sim_require_finite=True,
            sim_require_nnan=True,
            nc=nc,
        )
        return tuple(outs)

    devices = jax.devices()[:n_cores]
    mesh = Mesh(np.asarray(devices), ("core",))
    sharding = NamedSharding(mesh, PartitionSpec("core"))

    def make_global(shards):
        s0 = np.asarray(shards[0])
        gshape = (n_cores * s0.shape[0], *s0.shape[1:])
        parts = [
            jax.device_put(np.ascontiguousarray(shards[c]), devices[c])
            for c in range(n_cores)
        ]
        return jax.make_array_from_single_device_arrays(gshape, sharding, parts)

    global_ins = [make_global([m[nm] for m in in_maps]) for nm in in_names]
    global_zeros = [
        make_global([np.zeros(shape, dtype)] * n_cores)
        for shape, dtype in zero_shapes
    ]
    # force H2D completion so the NEFF's DMAs don't contend with PCIe-in
    for a in (*global_ins, *global_zeros):
        jax.block_until_ready(a)

    sharded = jax.jit(
        shard_map(_body, mesh=mesh, in_specs=(PartitionSpec("core"),) * (n_params + n_outs),
                  out_specs=(PartitionSpec("core"),) * n_outs, check_rep=False),
        donate_argnums=donate,
        keep_unused=True,
    )
    out_arrs = sharded(*global_ins, *global_zeros)

    results = [dict() for _ in range(n_cores)]
    for i, name in enumerate(out_names):
        arr = out_arrs[i]
        per = {s.index[0].start or 0: np.asarray(s.data) for s in arr.addressable_shards}
        step = out_avals[i].shape[0]
        for c in range(n_cores):
            results[c][name] = per[c * step]
    return results


def _patched_run_bass_via_pjrt(nc, in_maps, n_cores):
    if n_cores > 1 and nc.partition_id_tensor is None and nc.dbg_addr is None:
        return _run_bass_via_pjrt_presharded(nc, in_maps, n_cores)
    return _orig_run_bass_via_pjrt(nc, in_maps, n_cores)


_orig_run_bass_via_pjrt = _b2j.run_bass_via_pjrt
_b2j.run_bass_via_pjrt = _patched_run_bass_via_pjrt

P = 128
N = 577
C = 768
H = 12
D = 64
NCH = 5           # n (token) chunks: 4*128 + 65
CCH = 6           # c chunks: 768 / 128
NTAIL = N - 4 * P  # 65
F0, F1 = 290, 288  # n free-dim halves, padded n=578: fp32r needs EVEN free sizes
HALVES = ((0, F0), (F0, F1))
ITEMS = 8
NCORES = 8

f32 = mybir.dt.float32
f32r = mybir.dt.float32r
bf16 = mybir.dt.bfloat16


def _rows(nch):
    return NTAIL if nch == NCH - 1 else P


def _mcols(nch):
    """lhsT column count for an n-chunk: pad the 65-tail to 66 (even M is
    measurably faster on the PE); the extra output partition is discarded."""
    return NTAIL + 1 if nch == NCH - 1 else P


def _emit(ctx, tc, x_ext, wq_ext, pw_ext, pb_ext, sel_ext, out_ext):
    nc = tc.nc

    const_pool = ctx.enter_context(tc.tile_pool(name="const", bufs=1))
    wpool = ctx.enter_context(tc.tile_pool(name="weights", bufs=1))
    xtpool = ctx.enter_context(tc.tile_pool(name="xt", bufs=2))
    qkpool = ctx.enter_context(tc.tile_pool(name="qkt", bufs=2))
    vpool = ctx.enter_context(tc.tile_pool(name="vnat", bufs=2))
    epool = ctx.enter_context(tc.tile_pool(name="exps", bufs=4))
    aopool = ctx.enter_context(tc.tile_pool(name="aot", bufs=2))
    ypool = ctx.enter_context(tc.tile_pool(name="ychunk", bufs=2))
    spool = ctx.enter_context(tc.tile_pool(name="norm", bufs=2))
    aoupool = ctx.enter_context(tc.tile_pool(name="aou", bufs=2))
    # PSUM: 3x 2-bank slots (scores pairs + AV out) + 2x 1-bank slots
    # (qkv / proj / bcast staging) = 8 banks.
    ps2 = ctx.enter_context(tc.tile_pool(name="ps2", bufs=3, space="PSUM"))
    ps1 = ctx.enter_context(tc.tile_pool(name="ps1", bufs=2, space="PSUM"))

    def ps2_tile(name):
        return ps2.tile([P, 2, 512], f32, tag="ps2", name=name)

    def ps1_tile(name):
        return ps1.tile([P, 512], f32, tag="ps1", name=name)

    # ---- constants / weights (resident) ----
    ones_f32 = const_pool.tile([1, P], f32, name="ones_f32")
    nc.gpsimd.memset(ones_f32[:], 1.0)
    ones_row = const_pool.tile([1, P], f32r, name="ones_row")
    nc.vector.tensor_copy(ones_row[:], ones_f32[:])

    # head-broadcast selectors: hb=0 at partitions 0:6, hb=1 at 32:38 so the
    # two broadcast matmuls of a large item land in distinct PE row groups
    # and run concurrently.
    selA = const_pool.tile([6, CCH // 2, P], bf16, name="selA")
    nc.sync.dma_start(selA[:], sel_ext[0])
    selB = const_pool.tile([6, CCH // 2, P], bf16, name="selB")
    nc.sync.dma_start(selB[:], sel_ext[1])

    wq_sb = wpool.tile([P, CCH, 3 * C], bf16, name="wq_sb")
    nc.sync.dma_start(wq_sb[:], wq_ext.rearrange("(co p) o -> p co o", p=P))
    pw_sb = wpool.tile([P, CCH, C], bf16, name="pw_sb")
    nc.sync.dma_start(pw_sb[:], pw_ext.rearrange("(co p) o -> p co o", p=P))
    pb_sb = const_pool.tile([1, C], f32r, name="pb_sb")
    nc.sync.dma_start(pb_sb[:], pb_ext[None, :])

    # bias broadcast across partitions: [128, 768] = ones[128,1] @ pb[1,768]
    bias_sb = wpool.tile([P, C], f32, name="bias_sb")
    for j in range(2):
        psb0 = ps1_tile("ps_bias")
        nc.tensor.matmul(
            psb0[:, 0:384],
            lhsT=ones_row[0:1, :],
            rhs=pb_sb[0:1, j * 384:(j + 1) * 384],
            start=True, stop=True,
        )
        nc.vector.tensor_copy(bias_sb[:, j * 384:(j + 1) * 384], psb0[:, 0:384])

    # ---- per-item B/C phase, cut into PE filler groups ----
    def qk_group(st, oc):
        xT, qkT = st["xT"], st["qkT"]
        wcol = oc * P if oc < 6 else C + (oc - 6) * P
        for j, (n0, nsz) in enumerate(HALVES):
            pqk = ps1_tile("ps_qk")
            for cc in range(CCH):
                nc.tensor.matmul(
                    pqk[:, 0:nsz],
                    lhsT=wq_sb[:, cc, wcol:wcol + P],
                    rhs=xT[:, cc, n0:n0 + nsz],
                    start=(cc == 0), stop=(cc == CCH - 1),
                )
            nc.vector.tensor_copy(qkT[:, oc, n0:n0 + nsz], pqk[:, 0:nsz])

    def v_group(st, nch):
        xT, v_nat = st["xT"], st["v_nat"]
        rows = _rows(nch)
        for j in range(st["NJ"]):
            vcol = 2 * C + j * 384
            pv = ps1_tile("ps_v")
            for cc in range(CCH):
                nc.tensor.matmul(
                    pv[0:_mcols(nch), 0:384],
                    lhsT=xT[:, cc, nch * P:nch * P + _mcols(nch)],
                    rhs=wq_sb[:, cc, vcol:vcol + 384],
                    start=(cc == 0), stop=(cc == CCH - 1),
                )
            nc.vector.tensor_copy(
                v_nat[0:rows, nch, j * 6:(j + 1) * 6, 0:D],
                pv[0:rows, 0:384].rearrange("p (h d) -> p h d", h=6),
            )

    def make_item(it):
        small = it >= ITEMS // 2
        st = {
            "it": it,
            "small": small,
            "NJ": 1 if small else 2,
            "qk_chunks": [0, 1, 2, 6, 7, 8] if small else list(range(12)),
        }
        st["xT"] = xtpool.tile([P, CCH, 640], bf16, name="xT")
        nc.gpsimd.memset(st["xT"][:, :, N], 0.0)
        nc.sync.dma_start(
            st["xT"][:, :, 0:N],
            x_ext[it].rearrange("(co p) n -> p co n", p=P))
        st["qkT"] = qkpool.tile([P, 12, N + 1], bf16, name="qkT")
        st["v_nat"] = vpool.tile([P, NCH, H, 2 * D], bf16, name="v_nat")
        nc.gpsimd.memset(st["v_nat"][:, :, :, D:2 * D], 0.0)
        nc.gpsimd.memset(st["v_nat"][:, :, :, D], 1.0)
        groups = [lambda oc=oc: qk_group(st, oc) for oc in st["qk_chunks"]]
        groups += [lambda nch=nch: v_group(st, nch) for nch in range(NCH)]
        st["groups"] = groups
        return st

    fillq = deque()

    def filler(n=1):
        for _ in range(n):
            if fillq:
                fillq.popleft()()

    # ---- tail: normalize + output projection of a finished item ----
    def emit_tail(st):
        aoT, aoU, drecips, it, small = (st["aoT"], st["aoU"], st["drecips"],
                                        st["it"], st["small"])
        Heff = H // 2 if small else H
        CCH_ao = CCH // 2 if small else CCH
        NJ = 1 if small else 2
        # interleave hb0/hb1 chunks so the K=6 broadcast matmuls alternate
        # PE row groups (0 and 32) and pair up in the array.
        for c in range(Heff // 2):
            hb, cw = divmod(c, CCH // 2)
            base = 0
            selh = selA if hb == 0 else selB
            drecip = drecips[hb]
            for j, (n0, nsz) in enumerate(HALVES):
                pbc = ps1_tile("ps_bc")
                nc.tensor.matmul(
                    pbc[:, 0:nsz],
                    lhsT=selh[base:base + 6, cw, :],
                    rhs=drecip[base:base + 6, j, 0:nsz],
                    start=True, stop=True,
                )
                nc.vector.tensor_mul(
                    aoT[:, c, n0:n0 + nsz],
                    aoU[:, c, j, 0:nsz],
                    pbc[:, 0:nsz],
                )
        for nch in range(NCH):
            rows = _rows(nch)
            yc = ypool.tile([P, C], f32, name="yc")
            if small:
                nc.gpsimd.memset(yc[0:rows, 384:768], 0.0)
            for j in range(NJ):
                o0 = j * 384
                psy = ps1_tile("ps_y")
                for cc in range(CCH_ao):
                    nc.tensor.matmul(
                        psy[0:_mcols(nch), 0:384],
                        lhsT=aoT[:, cc, nch * P:nch * P + _mcols(nch)],
                        rhs=pw_sb[:, cc, o0:o0 + 384],
                        start=(cc == 0), stop=(cc == CCH_ao - 1),
                    )
                nc.vector.tensor_add(
                    yc[0:rows, o0:o0 + 384],
                    psy[0:rows, 0:384],
                    bias_sb[0:rows, o0:o0 + 384],
                )
            nc.sync.dma_start(out_ext[it, nch * P:nch * P + rows, :],
                              yc[0:rows, :])
            filler()

    # ---- D/E phase: scores + exp + AV, with PE filler interleaved ----
    def emit_de(st, tail_st):
        small = st["small"]
        qkT, v_nat = st["qkT"], st["v_nat"]
        Heff = H // 2 if small else H

        aoT = aopool.tile([P, CCH, N + 1], bf16, name="aoT")
        aoU = aoupool.tile([P, CCH, 2, F0], bf16, name="aoU")
        dsums = [spool.tile([6, 2, F0], f32, tag="dsum", name="dsum0")]
        if Heff == H:
            dsums.append(spool.tile([6, 2, F0], f32, tag="dsum1",
                                    name="dsum1"))
        drecips = [None, None]
        st["aoT"], st["aoU"], st["drecips"] = aoT, aoU, drecips

        def emit_scores(hp):
            kch = 6 + hp
            expS = [None, None]
            for hh in (0, 1):
                expS[hh] = epool.tile([P, NCH, 2, F0], bf16, tag="expS",
                                      name=f"expS{hh}")
            for mch in range(NCH):
                mrows = _rows(mch)
                pse = [ps2_tile("ps_s0"), ps2_tile("ps_s1")]
                for j, (n0, nsz) in enumerate(HALVES):
                    for hh in (0, 1):
                        # adjacent issue of disjoint row-groups -> the two
                        # K=64 matmuls run concurrently in the PE array
                        hrow = hh * D
                        nc.tensor.matmul(
                            pse[hh][0:_mcols(mch), j, 0:nsz],
                            lhsT=qkT[hrow:hrow + D, kch, mch * P:mch * P + _mcols(mch)],
                            rhs=qkT[hrow:hrow + D, hp, n0:n0 + nsz],
                            start=True, stop=True,
                            tile_position=(hrow, 0),
                        )
                for hh in (0, 1):
                    nc.scalar.activation(
                        expS[hh][0:mrows, mch, :, :],
                        pse[hh][0:mrows, :, 0:F0],
                        mybir.ActivationFunctionType.Exp,
                    )
                filler()
            return expS

        def emit_av(hp, expS):
            for hh in (0, 1):
                h = 2 * hp + hh
                po = ps2_tile("ps_o")
                for j, (n0, nsz) in enumerate(HALVES):
                    for mch in range(NCH):
                        mrows = _rows(mch)
                        nc.tensor.matmul(
                            po[:, j, 0:nsz],
                            lhsT=v_nat[0:mrows, mch, h, :],
                            rhs=expS[hh][0:mrows, mch, j, 0:nsz],
                            start=(mch == 0), stop=(mch == NCH - 1),
                        )
                arow = hh * D
                nc.vector.tensor_copy(aoU[arow:arow + D, hp, :, :],
                                      po[0:D, :, 0:F0])
                dstage = spool.tile([1, 2, F0], f32, name="dstage")
                nc.vector.tensor_copy(dstage[0:1, :, :], po[D:D + 1, :, 0:F0])
                hb, hw = divmod(h, 6)
                nc.gpsimd.dma_start(dsums[hb][hw:hw + 1, :, :],
                                    dstage[0:1, :, :])
                filler()

        def emit_recip(hb):
            scr = spool.tile([6, 2, F0], f32, tag=f"rscr{hb}", name=f"rscr{hb}")
            nc.vector.reciprocal_approx_fast(
                scr[0:6, :, :], dsums[hb][0:6, :, :])
            dr = spool.tile([6, 2, F0], bf16, tag=f"drecip{hb}",
                            name=f"drecip{hb}")
            nc.vector.tensor_copy(dr[0:6, :, :], scr[0:6, :, :])
            drecips[hb] = dr

        prev = None
        for hp in range(Heff // 2):
            expS = emit_scores(hp)
            if hp == 0 and tail_st is not None:
                emit_tail(tail_st)      # prev item's normalize+proj: PE filler
            if prev is not None:
                emit_av(prev[0], prev[1])
                if prev[0] == 2:
                    emit_recip(0)       # heads 0..5 all staged
            prev = (hp, expS)
        emit_av(prev[0], prev[1])
        if Heff == H:
            emit_recip(1)
        else:
            emit_recip(0)

    # ---- driver: 2-deep software pipeline over items ----
    import os
    no_ilv = bool(os.environ.get("KERNEL_NO_INTERLEAVE"))
    nxt = make_item(0)
    fillq.extend(nxt["groups"])
    filler(len(fillq))                  # item 0's B/C has nothing to hide under
    pending = None
    for it in range(ITEMS):
        cur = nxt
        filler(len(fillq))              # finish any leftover B/C of cur
        if it + 1 < ITEMS:
            nxt = make_item(it + 1)
            fillq.extend(nxt["groups"])
            if no_ilv:
                filler(len(fillq))
        emit_de(cur, pending)           # B/C of it+1 fills exp-bound stretches
        pending = cur
    filler(len(fillq))
    emit_tail(pending)


_GRAPH = None


def _get_graph():
    global _GRAPH
    if _GRAPH is None:
        nc = bacc.Bacc("TRN2", target_bir_lowering=False, debug=False,
                       num_devices=NCORES)
        x_ext = nc.dram_tensor("x", [ITEMS, C, N], bf16, kind="ExternalInput").ap()
        wq_ext = nc.dram_tensor("wq", [C, 3 * C], bf16, kind="ExternalInput").ap()
        pw_ext = nc.dram_tensor("pw", [C, C], bf16, kind="ExternalInput").ap()
        pb_ext = nc.dram_tensor("pb", [C], f32r, kind="ExternalInput").ap()
        sel_ext = nc.dram_tensor("sel", [2, 6, CCH // 2, P], bf16, kind="ExternalInput").ap()
        out_ext = nc.dram_tensor("out", [ITEMS, N, C], f32, kind="ExternalOutput").ap()
        with tile.TileContext(nc) as tc:
            with ExitStack() as ctx:
                _emit(ctx, tc, x_ext, wq_ext, pw_ext, pb_ext, sel_ext, out_ext)
        nc.finalize()
        _GRAPH = nc
    return _GRAPH


LAST_RESULTS = None


def kernel(x, qkv_w, proj_w, proj_b, _trace=False):
    global LAST_RESULTS
    x = np.asarray(x, dtype=np.float32)
    xT_all = np.ascontiguousarray(x.transpose(0, 2, 1)).astype(ml_dtypes.bfloat16)
    wq = np.array(qkv_w, dtype=np.float32)          # copy; rows 0:C are q
    wq[0:C] *= D ** -0.5                            # fold attention scale into Wq
    wqT = np.ascontiguousarray(wq.T).astype(ml_dtypes.bfloat16)   # [C, 3C]
    pwT = np.ascontiguousarray(
        np.asarray(proj_w, dtype=np.float32).T).astype(ml_dtypes.bfloat16)
    pb = np.ascontiguousarray(np.asarray(proj_b, dtype=np.float32))
    sel_np = np.zeros((2, 6, CCH // 2, P), dtype=ml_dtypes.bfloat16)
    for c in range(CCH):
        hb, cw = divmod(c, CCH // 2)
        sel_np[hb, 2 * cw, cw, 0:D] = 1
        sel_np[hb, 2 * cw + 1, cw, D:P] = 1

    nc = _get_graph()
    in_maps = []
    half = x.shape[0] // 2  # 32
    per = half // NCORES    # 4
    for c in range(NCORES):
        xs = np.concatenate(
            [xT_all[per * c:per * (c + 1)],
             xT_all[half + per * c:half + per * (c + 1)]],
            axis=0,
        )
        in_maps.append({
            "x": np.ascontiguousarray(xs),
            "wq": wqT,
            "pw": pwT,
            "pb": pb,
            "sel": sel_np,
        })

    res = run_bass_kernel_spmd(nc, in_maps, core_ids=list(range(NCORES)),
                               trace=_trace)
    LAST_RESULTS = res

    out = np.empty((x.shape[0], N, C), dtype=np.float32)
    for c in range(NCORES):
        o = res.results[c]["out"]
        out[per * c:per * (c + 1)] = o[0:per]
        out[half + per * c:half + per * (c + 1)] = o[per:2 * per]
    return out
